# revision 29
# baseline (speedup 1.0000x reference)
# Multi-head attention (B=2, S=2048, D=1024, H=16) on 8 Trainium2 NeuronCores.
#
# Sharding: core c handles batch b = c // 4 and heads [4*(c%4), 4*(c%4)+4).
# Host pre-transposes activations to [D, S] (matmul contracts over the
# partition axis, so activations must arrive transposed), pre-slices the
# per-head weight columns, folds the 1/sqrt(dk) score scale into W_q/b_q,
# and sums the 4 partial output projections per batch (+ b_o) at the end.
#
# On-device dataflow per core (all matmuls bf16, 1 cyc/row):
#   P:  QhT = W_q.T @ q.T  (heads packed in pairs -> [128, S] tiles),
#       KhT likewise, vh = v.T.T @ W_v in natural [S, dk] layout with an
#       appended ones column (gives attention row-sums for free in PV).
#       Biases are added with K=1 rank-1 matmuls (ones ⊗ bias).
#   A:  scores[q,k] tiles on PE (K=64 row-group packing runs the two heads
#       of a pair concurrently), exp on ACT with accum_out row sums,
#       reciprocal + per-partition scale on DVE, DMA fp32 attn out.
#   B:  scores^T[k,q] tiles on PE (same inputs, swapped lhsT/rhs -- no
#       transpose pass needed), exp on ACT (bf16), PV matmul accumulates
#       ctx^T[d,q] with a sums row, normalization via gpsimd
#       partition-broadcast of the reciprocal sums row.
#   O:  out^T partial = ctx^T.T @ W_o chunk-accumulated, DMA from PSUM.

import numpy as np
import ml_dtypes

B, S, D, H, DK = 2, 2048, 1024, 16, 64
HPC = H // 4  # heads per core (4)
NCORES = 8

_BF16 = ml_dtypes.bfloat16
_CACHE = {}


def _build_nc(s=S):
    import concourse.bacc as bacc
    import concourse.mybir as mybir
    import concourse.tile as tile

    dt = mybir.dt
    f32, bf16 = dt.float32, dt.bfloat16
    AF = mybir.ActivationFunctionType

    EC = D // 128        # contraction chunks for projections (8)
    QT = s // 128        # 128-row q/k tiles (16)
    NJ = s // 512        # 512-col tiles (4)
    WB = 512 if s >= 512 else s     # phase-B q-slab width
    NH = s // WB         # number of q slabs in phase B
    J2 = WB // 512       # 512-col tiles per slab

    nc = bacc.Bacc("TRN2", target_bir_lowering=False, debug=False,
                   num_devices=NCORES)

    qT = nc.dram_tensor("qT", [D, s], bf16, kind="ExternalInput")
    kT = nc.dram_tensor("kT", [D, s], bf16, kind="ExternalInput")
    vT = nc.dram_tensor("vT", [D, s], bf16, kind="ExternalInput")
    wq = nc.dram_tensor("wq", [D, HPC * DK], bf16, kind="ExternalInput")
    wk = nc.dram_tensor("wk", [D, HPC * DK], bf16, kind="ExternalInput")
    wv = nc.dram_tensor("wv", [D, HPC * DK], bf16, kind="ExternalInput")
    wo = nc.dram_tensor("wo", [HPC * DK, D], bf16, kind="ExternalInput")
    bq = nc.dram_tensor("bq", [1, HPC * DK], bf16, kind="ExternalInput")
    bk = nc.dram_tensor("bk", [1, HPC * DK], bf16, kind="ExternalInput")
    bv = nc.dram_tensor("bv", [1, HPC * DK], bf16, kind="ExternalInput")
    attn_d = nc.dram_tensor("attn", [HPC, s, s], bf16, kind="ExternalOutput")
    outp_d = nc.dram_tensor("outp", [s, D], bf16, kind="ExternalOutput")

    with tile.TileContext(nc) as tc:
        with tc.tile_pool(name="pers", bufs=1) as pers:
            qhT = pers.tile([128, 2, s], bf16, tag="qhT")
            khT = pers.tile([128, 2, s], bf16, tag="khT")
            vh = pers.tile([128, QT, HPC, DK + 1], bf16, tag="vh")
            ctxT = pers.tile([128, 2, s], bf16, tag="ctxT")
            wo_sb = pers.tile([128, 2, D], bf16, tag="wo")
            ones = pers.tile([1, 512], bf16, tag="ones")
            zros = pers.tile([1, 512], bf16, tag="zros")
            bq_sb = pers.tile([1, HPC * DK], bf16, tag="bq")
            bk_sb = pers.tile([1, HPC * DK], bf16, tag="bk")
            bv_sb = pers.tile([1, HPC * DK], bf16, tag="bv")

            nc.vector.memset(ones, 1.0)
            nc.vector.memset(zros, 0.0)

            def warm(ps, n, wide, gate=None):
                # Zero-adding keep-warm matmuls (lhsT = zeros, so the live
                # PSUM accumulator is unchanged). They execute during PE
                # idle gaps so the HAM clock gate never sees a fully-idle
                # window and the PE stays at 2.4 GHz. A dummy emitted after
                # a consumer of its target region is WAR-ordered behind that
                # consumer, which time-spaces it into the idle gap.
                rhs = ones[0:1, 0:wide]
                for _ in range(n):
                    nc.tensor.matmul(ps[:, 0:wide], zros[:, 0:128], rhs,
                                     start=False, stop=False,
                                     skip_group_check=True)
            nc.vector.memset(vh[:, :, :, DK : DK + 1], 1.0)
            nc.sync.dma_start(wo_sb, wo.rearrange("(c p) e -> p c e", p=128))
            nc.sync.dma_start(bq_sb, bq[:, :])
            nc.sync.dma_start(bk_sb, bk[:, :])
            nc.sync.dma_start(bv_sb, bv[:, :])

            # ---------------- phase P: projections ----------------
            with tc.tile_pool(name="xin", bufs=1) as xin, \
                 tc.tile_pool(name="wio", bufs=1) as wio:
                qT_sb = xin.tile([128, EC, s], bf16, tag="qT")
                kT_sb = xin.tile([128, EC, s], bf16, tag="kT")
                vT_sb = xin.tile([128, EC, s], bf16, tag="vT")
                wq_sb = wio.tile([128, EC, HPC * DK], bf16, tag="wq")
                wk_sb = wio.tile([128, EC, HPC * DK], bf16, tag="wk")
                wv_sb = wio.tile([128, EC, HPC * DK], bf16, tag="wv")
                nc.sync.dma_start(wq_sb, wq.rearrange("(c p) n -> p c n", p=128))
                nc.sync.dma_start(wk_sb, wk.rearrange("(c p) n -> p c n", p=128))
                nc.sync.dma_start(wv_sb, wv.rearrange("(c p) n -> p c n", p=128))
                qT_v = qT.rearrange("(c p) ss -> c p ss", p=128)
                kT_v = kT.rearrange("(c p) ss -> c p ss", p=128)
                vT_v = vT.rearrange("(c p) ss -> c p ss", p=128)
                for c in range(EC):
                    nc.sync.dma_start(qT_sb[:, c, :], qT_v[c])
                    nc.sync.dma_start(kT_sb[:, c, :], kT_v[c])
                    nc.sync.dma_start(vT_sb[:, c, :], vT_v[c])

                # QhT / KhT, packed two heads per 128 partitions.
                # e-chunk-outer with 8 persistent PSUM banks: each input
                # chunk is consumed right after its DMA lands and the PE
                # stream stays dense.
                with tc.tile_pool(name="ppqk", bufs=2 * NJ, space="PSUM") as ppqk:
                  for xi, (x_sb, w_sb, b_sb, dst) in enumerate((
                    (qT_sb, wq_sb, bq_sb, qhT),
                    (kT_sb, wk_sb, bk_sb, khT),
                  )):
                    pss = [ppqk.tile([128, 512], f32, tag="pp",
                                     name=f"pp{xi}_{i}")
                           for i in range(2 * NJ)]
                    if xi == 0:
                        # HAM warm-up: ~10us of dependency-free matmuls runs
                        # while the input DMAs stream (PE would idle here),
                        # flipping the PE clock gate to 2.4 GHz for the rest
                        # of the kernel.
                        for _ in range(24):
                            nc.tensor.matmul(pss[0], ones[:, 0:128],
                                             ones[:, :],
                                             start=True, stop=True)
                    for c in range(EC):
                        for pair in range(2):
                            pc = slice(pair * 128, (pair + 1) * 128)
                            for j in range(NJ):
                                js = slice(j * 512, (j + 1) * 512)
                                nc.tensor.matmul(pss[pair * NJ + j],
                                                 w_sb[:, c, pc],
                                                 x_sb[:, c, js],
                                                 start=(c == 0), stop=False)
                    for pair in range(2):
                        pc = slice(pair * 128, (pair + 1) * 128)
                        for j in range(NJ):
                            js = slice(j * 512, (j + 1) * 512)
                            ps = pss[pair * NJ + j]
                            nc.tensor.matmul(ps, b_sb[:, pc], ones[:, 0:512],
                                             start=False, stop=True)
                            nc.vector.tensor_copy(dst[:, pair, js], ps)

                # vh in natural [s, d] layout (+ ones column already set)
                with tc.tile_pool(name="ppv", bufs=3, space="PSUM") as ppv:
                  for st in range(QT):
                    ss = slice(st * 128, (st + 1) * 128)
                    ps = ppv.tile([128, HPC * DK], f32, tag="vp")
                    for c in range(EC):
                        nc.tensor.matmul(ps, vT_sb[:, c, ss], wv_sb[:, c, :],
                                         start=(c == 0), stop=False)
                    nc.tensor.matmul(ps, ones[:, 0:128], bv_sb[:, :],
                                     start=False, stop=True)
                    nc.vector.tensor_copy(
                        vh[:, st, :, 0:DK],
                        ps.rearrange("p (h d) -> p h d", h=HPC))

            # ---------------- phases A and B per head pair ----------------
            for pair in range(2):
                # phase A: scores [q, k], exp + sums, scale, writeout
                with tc.tile_pool(name=f"scA{pair}", bufs=2, space="PSUM") as scA, \
                     tc.tile_pool(name=f"exA{pair}", bufs=3) as exA, \
                     tc.tile_pool(name=f"wrA{pair}", bufs=3) as wrA, \
                     tc.tile_pool(name=f"rcA{pair}", bufs=8) as rcA:
                    for qt in range(QT):
                        qs = slice(qt * 128, (qt + 1) * 128)
                        pss = [scA.tile([128, s], f32, tag="sc",
                                        name=f"sc{qt}_{hh}")
                               for hh in range(2)]
                        for j in range(NJ):
                            js = slice(j * 512, (j + 1) * 512)
                            for hh in range(2):
                                hp = slice(hh * 64, (hh + 1) * 64)
                                nc.tensor.matmul(
                                    pss[hh][:, js],
                                    qhT[hp, pair, qs], khT[hp, pair, js],
                                    start=True, stop=True)
                        if qt == 0:
                            warm(pss[1], 8, 512)
                        for hh in range(2):
                            h = 2 * pair + hh
                            ex = exA.tile([128, s], f32, tag="ex")
                            sm = rcA.tile([128, 1], f32, tag="sm")
                            nc.scalar.activation(ex, pss[hh], AF.Exp,
                                                 accum_out=sm)
                            rc = rcA.tile([128, 1], f32, tag="rc")
                            nc.vector.reciprocal_approx_fast(rc, sm)
                            wr = wrA.tile([128, s], bf16, tag="wr")
                            nc.vector.tensor_scalar_mul(wr, ex, rc)
                            nc.sync.dma_start(attn_d[h, qs, :], wr)

                # phase B: scores^T [k, q], exp, PV with sums row, ctxT
                with tc.tile_pool(name=f"scB{pair}", bufs=2, space="PSUM") as scB, \
                     tc.tile_pool(name=f"pvB{pair}", bufs=4 * J2, space="PSUM") as pvB, \
                     tc.tile_pool(name=f"exB{pair}", bufs=4) as exB, \
                     tc.tile_pool(name=f"rbB{pair}", bufs=4) as rbB:
                    for hf in range(NH):
                        pv = [[[pvB.tile([DK + 1, 512], f32, tag="pv",
                                         name=f"pv{hf}_{hh}_{half}_{j2}")
                                for j2 in range(J2)] for half in range(2)]
                              for hh in range(2)]

                        # software-pipelined emission: PE's strict FIFO must
                        # never block on an exp -- scores^T mms for k-tile kt
                        # are emitted before the PV mms of k-tile kt-1, and
                        # the two heads' mms are adjacent (disjoint PE
                        # row-groups run concurrently).
                        def _emit_pv(kt):
                            # contraction split into 64-row halves: the four
                            # matmuls per k-tile alternate PE row-groups and
                            # write four DISTINCT PSUM banks, so they run
                            # concurrently (row-group packing) and LDWEIGHTS
                            # pulls ahead. Halves are merged at copy-out.
                            for j2 in range(J2):
                                js = slice(j2 * 512, (j2 + 1) * 512)
                                for step in range(2):
                                    for hh in range(2):
                                        half = step ^ hh
                                        rp = slice(half * 64, (half + 1) * 64)
                                        h = 2 * pair + hh
                                        nc.tensor.matmul(
                                            pv[hh][half][j2],
                                            vh[rp, kt, h, :],
                                            exs[kt % 2][hh][rp, js],
                                            start=(kt == 0),
                                            stop=(kt == QT - 1))

                        exs = [None, None]
                        for kt in range(QT):
                            ks = slice(kt * 128, (kt + 1) * 128)
                            # both heads share one [128, 1024] tile: the two
                            # row-group-packed matmuls land in different PSUM
                            # banks (cols 0-511 / 512-1023) and stay
                            # concurrent, and ONE exp covers both heads.
                            scT = scB.tile([128, 2 * WB], f32, tag="scT",
                                           name=f"scT{hf}_{kt}")
                            qcol = slice(hf * WB, (hf + 1) * WB)
                            for hh in range(2):
                                hp = slice(hh * 64, (hh + 1) * 64)
                                nc.tensor.matmul(
                                    scT[:, hh * WB : (hh + 1) * WB],
                                    khT[hp, pair, ks], qhT[hp, pair, qcol],
                                    start=True, stop=True)
                            if kt == 0 and hf == 0:
                                warm(scT, 8, 512)
                            if kt > 0:
                                _emit_pv(kt - 1)
                            ex = exB.tile([128, 2 * WB], bf16, tag="ex",
                                          name=f"ex{hf}_{kt}")
                            nc.scalar.activation(ex, scT, AF.Exp)
                            exs[kt % 2] = [ex[:, 0:WB], ex[:, WB : 2 * WB]]
                        _emit_pv(QT - 1)

                        # copy PV accumulators out of PSUM promptly (frees the
                        # banks for the next slab), then normalize from SBUF
                        for hh in range(2):
                            hp = slice(hh * 64, (hh + 1) * 64)
                            for j2 in range(J2):
                                qcol = slice(hf * WB + j2 * 512,
                                             hf * WB + (j2 + 1) * 512)
                                h0c = rbB.tile([DK + 1, 512], f32,
                                               tag="h0c")
                                nc.vector.tensor_copy(h0c, pv[hh][0][j2])
                                cp = rbB.tile([DK, 512], f32, tag="cp")
                                nc.vector.tensor_add(
                                    cp, h0c[0:DK, :], pv[hh][1][j2][0:DK, :])
                                srow = rbB.tile([1, 512], f32, tag="srow")
                                nc.vector.tensor_add(
                                    srow, h0c[DK : DK + 1, :],
                                    pv[hh][1][j2][DK : DK + 1, :])
                                sb = rbB.tile([DK, 512], f32, tag="sb")
                                nc.gpsimd.partition_broadcast(sb, srow)
                                rb = rbB.tile([DK, 512], f32, tag="rb")
                                nc.vector.reciprocal_approx_fast(rb, sb)
                                nc.vector.tensor_mul(
                                    ctxT[hp, pair, qcol], cp, rb)

            # ---------------- phase O: output projection ----------------
            with tc.tile_pool(name="oP", bufs=4, space="PSUM") as oP, \
                 tc.tile_pool(name="oS", bufs=4) as oS:
                for st in range(QT):
                    ss = slice(st * 128, (st + 1) * 128)
                    for ej in range(D // 512):
                        es = slice(ej * 512, (ej + 1) * 512)
                        po = oP.tile([128, 512], f32, tag="po")
                        for c in range(2):
                            nc.tensor.matmul(po, ctxT[:, c, ss],
                                             wo_sb[:, c, es],
                                             start=(c == 0), stop=(c == 1))
                        ob = oS.tile([128, 512], bf16, tag="ob")
                        nc.vector.tensor_copy(ob, po)
                        nc.sync.dma_start(outp_d[ss, es], ob)

    nc.compile()
    return nc


def _get_nc(s=S):
    if s not in _CACHE:
        _CACHE[s] = _build_nc(s)
    return _CACHE[s]


def kernel(q, k, v, W_q, b_q, W_k, b_k, W_v, b_v, W_o, b_o):
    from concourse.bass_utils import run_bass_kernel_spmd

    q = np.asarray(q, np.float32)
    k = np.asarray(k, np.float32)
    v = np.asarray(v, np.float32)
    s = q.shape[1]
    nc = _get_nc(s)

    scale = np.float32(1.0 / np.sqrt(DK))
    # per-batch transposed activations (shared by the 4 cores of a batch)
    qT = [np.ascontiguousarray(q[b].T).astype(_BF16) for b in range(B)]
    kT = [np.ascontiguousarray(k[b].T).astype(_BF16) for b in range(B)]
    vT = [np.ascontiguousarray(v[b].T).astype(_BF16) for b in range(B)]

    in_maps = []
    for c in range(NCORES):
        b, hg = c // 4, c % 4
        cols = slice(hg * HPC * DK, (hg + 1) * HPC * DK)
        in_maps.append({
            "qT": qT[b], "kT": kT[b], "vT": vT[b],
            "wq": np.ascontiguousarray(W_q[:, cols] * scale).astype(_BF16),
            "wk": np.ascontiguousarray(W_k[:, cols]).astype(_BF16),
            "wv": np.ascontiguousarray(W_v[:, cols]).astype(_BF16),
            "wo": np.ascontiguousarray(W_o[cols, :]).astype(_BF16),
            "bq": (b_q[cols] * scale).astype(_BF16).reshape(1, -1),
            "bk": b_k[cols].astype(_BF16).reshape(1, -1),
            "bv": b_v[cols].astype(_BF16).reshape(1, -1),
        })

    res = run_bass_kernel_spmd(nc, in_maps, core_ids=list(range(NCORES)))

    attn = np.empty((B, H, s, s), np.float32)
    out = np.zeros((B, s, D), np.float32)
    for c in range(NCORES):
        b, hg = c // 4, c % 4
        attn[b, hg * HPC : (hg + 1) * HPC] = res.results[c]["attn"].astype(np.float32)
        out[b] += res.results[c]["outp"].astype(np.float32)
    out += b_o.astype(np.float32)
    return out, attn


# revision 30
# speedup vs baseline: 1.0251x; 1.0251x over previous
# Multi-head attention (B=2, S=2048, D=1024, H=16) on 8 Trainium2 NeuronCores.
#
# Sharding: core c handles batch b = c // 4 and heads [4*(c%4), 4*(c%4)+4).
# Host pre-transposes activations to [D, S] (matmul contracts over the
# partition axis, so activations must arrive transposed), pre-slices the
# per-head weight columns, folds the 1/sqrt(dk) score scale into W_q/b_q,
# and sums the 4 partial output projections per batch (+ b_o) at the end.
#
# On-device dataflow per core (all matmuls bf16, 1 cyc/row):
#   P:  QhT = W_q.T @ q.T  (heads packed in pairs -> [128, S] tiles),
#       KhT likewise, vh = v.T.T @ W_v in natural [S, dk] layout with an
#       appended ones column (gives attention row-sums for free in PV).
#       Biases are added with K=1 rank-1 matmuls (ones ⊗ bias).
#   A:  scores[q,k] tiles on PE (K=64 row-group packing runs the two heads
#       of a pair concurrently), exp on ACT with accum_out row sums,
#       reciprocal + per-partition scale on DVE, DMA fp32 attn out.
#   B:  scores^T[k,q] tiles on PE (same inputs, swapped lhsT/rhs -- no
#       transpose pass needed), exp on ACT (bf16), PV matmul accumulates
#       ctx^T[d,q] with a sums row, normalization via gpsimd
#       partition-broadcast of the reciprocal sums row.
#   O:  out^T partial = ctx^T.T @ W_o chunk-accumulated, DMA from PSUM.

import numpy as np
import ml_dtypes

B, S, D, H, DK = 2, 2048, 1024, 16, 64
HPC = H // 4  # heads per core (4)
NCORES = 8

_BF16 = ml_dtypes.bfloat16
_CACHE = {}


def _build_nc(s=S):
    import concourse.bacc as bacc
    import concourse.mybir as mybir
    import concourse.tile as tile

    dt = mybir.dt
    f32, bf16 = dt.float32, dt.bfloat16
    AF = mybir.ActivationFunctionType

    EC = D // 128        # contraction chunks for projections (8)
    QT = s // 128        # 128-row q/k tiles (16)
    NJ = s // 512        # 512-col tiles (4)
    WB = 512 if s >= 512 else s     # phase-B q-slab width
    NH = s // WB         # number of q slabs in phase B
    J2 = WB // 512       # 512-col tiles per slab

    nc = bacc.Bacc("TRN2", target_bir_lowering=False, debug=False,
                   num_devices=NCORES)

    qT = nc.dram_tensor("qT", [D, s], bf16, kind="ExternalInput")
    kT = nc.dram_tensor("kT", [D, s], bf16, kind="ExternalInput")
    vT = nc.dram_tensor("vT", [D, s], bf16, kind="ExternalInput")
    wq = nc.dram_tensor("wq", [D, HPC * DK], bf16, kind="ExternalInput")
    wk = nc.dram_tensor("wk", [D, HPC * DK], bf16, kind="ExternalInput")
    wv = nc.dram_tensor("wv", [D, HPC * DK], bf16, kind="ExternalInput")
    wo = nc.dram_tensor("wo", [HPC * DK, D], bf16, kind="ExternalInput")
    bq = nc.dram_tensor("bq", [1, HPC * DK], bf16, kind="ExternalInput")
    bk = nc.dram_tensor("bk", [1, HPC * DK], bf16, kind="ExternalInput")
    bv = nc.dram_tensor("bv", [1, HPC * DK], bf16, kind="ExternalInput")
    attn_d = nc.dram_tensor("attn", [HPC, s, s], bf16, kind="ExternalOutput")
    outp_d = nc.dram_tensor("outp", [s, D], bf16, kind="ExternalOutput")

    with tile.TileContext(nc) as tc:
        with tc.tile_pool(name="pers", bufs=1) as pers:
            qhT = pers.tile([128, 2, s], bf16, tag="qhT")
            khT = pers.tile([128, 2, s], bf16, tag="khT")
            vh = pers.tile([128, QT, HPC, DK + 1], bf16, tag="vh")
            ctxT = pers.tile([128, 2, s], bf16, tag="ctxT")
            wo_sb = pers.tile([128, 2, D], bf16, tag="wo")
            ones = pers.tile([1, 512], bf16, tag="ones")
            zros = pers.tile([1, 512], bf16, tag="zros")
            bq_sb = pers.tile([1, HPC * DK], bf16, tag="bq")
            bk_sb = pers.tile([1, HPC * DK], bf16, tag="bk")
            bv_sb = pers.tile([1, HPC * DK], bf16, tag="bv")

            nc.vector.memset(ones, 1.0)
            nc.vector.memset(zros, 0.0)

            def warm(ps, n, wide, gate=None):
                # Zero-adding keep-warm matmuls (lhsT = zeros, so the live
                # PSUM accumulator is unchanged). They execute during PE
                # idle gaps so the HAM clock gate never sees a fully-idle
                # window and the PE stays at 2.4 GHz. A dummy emitted after
                # a consumer of its target region is WAR-ordered behind that
                # consumer, which time-spaces it into the idle gap.
                rhs = ones[0:1, 0:wide]
                for _ in range(n):
                    nc.tensor.matmul(ps[:, 0:wide], zros[:, 0:128], rhs,
                                     start=False, stop=False,
                                     skip_group_check=True)
            nc.vector.memset(vh[:, :, :, DK : DK + 1], 1.0)
            nc.sync.dma_start(wo_sb, wo.rearrange("(c p) e -> p c e", p=128))
            nc.sync.dma_start(bq_sb, bq[:, :])
            nc.sync.dma_start(bk_sb, bk[:, :])
            nc.sync.dma_start(bv_sb, bv[:, :])

            # ---------------- phase P: projections ----------------
            with tc.tile_pool(name="xin", bufs=1) as xin, \
                 tc.tile_pool(name="wio", bufs=1) as wio:
                qT_sb = xin.tile([128, EC, s], bf16, tag="qT")
                kT_sb = xin.tile([128, EC, s], bf16, tag="kT")
                vT_sb = xin.tile([128, EC, s], bf16, tag="vT")
                wq_sb = wio.tile([128, EC, HPC * DK], bf16, tag="wq")
                wk_sb = wio.tile([128, EC, HPC * DK], bf16, tag="wk")
                wv_sb = wio.tile([128, EC, HPC * DK], bf16, tag="wv")
                # per-chunk transfers, emitted in consumption order, so
                # the first projection matmuls start as soon as chunk 0 of
                # the weights + activations lands (not after 512KB monoliths)
                wq_v = wq.rearrange("(c p) n -> c p n", p=128)
                wk_v = wk.rearrange("(c p) n -> c p n", p=128)
                wv_v = wv.rearrange("(c p) n -> c p n", p=128)
                qT_v = qT.rearrange("(c p) ss -> c p ss", p=128)
                kT_v = kT.rearrange("(c p) ss -> c p ss", p=128)
                vT_v = vT.rearrange("(c p) ss -> c p ss", p=128)
                for c in range(EC):
                    nc.sync.dma_start(wq_sb[:, c, :], wq_v[c])
                    nc.sync.dma_start(qT_sb[:, c, :], qT_v[c])
                for c in range(EC):
                    nc.sync.dma_start(wk_sb[:, c, :], wk_v[c])
                    nc.sync.dma_start(kT_sb[:, c, :], kT_v[c])
                for c in range(EC):
                    nc.sync.dma_start(wv_sb[:, c, :], wv_v[c])
                    nc.sync.dma_start(vT_sb[:, c, :], vT_v[c])

                # QhT / KhT, packed two heads per 128 partitions.
                # e-chunk-outer with 8 persistent PSUM banks: each input
                # chunk is consumed right after its DMA lands and the PE
                # stream stays dense.
                with tc.tile_pool(name="ppqk", bufs=2 * NJ, space="PSUM") as ppqk:
                  for xi, (x_sb, w_sb, b_sb, dst) in enumerate((
                    (qT_sb, wq_sb, bq_sb, qhT),
                    (kT_sb, wk_sb, bk_sb, khT),
                  )):
                    pss = [ppqk.tile([128, 512], f32, tag="pp",
                                     name=f"pp{xi}_{i}")
                           for i in range(2 * NJ)]
                    if xi == 0:
                        # HAM warm-up: ~10us of dependency-free matmuls runs
                        # while the input DMAs stream (PE would idle here),
                        # flipping the PE clock gate to 2.4 GHz for the rest
                        # of the kernel.
                        for _ in range(24):
                            nc.tensor.matmul(pss[0], ones[:, 0:128],
                                             ones[:, :],
                                             start=True, stop=True)
                    for c in range(EC):
                        for pair in range(2):
                            pc = slice(pair * 128, (pair + 1) * 128)
                            for j in range(NJ):
                                js = slice(j * 512, (j + 1) * 512)
                                nc.tensor.matmul(pss[pair * NJ + j],
                                                 w_sb[:, c, pc],
                                                 x_sb[:, c, js],
                                                 start=(c == 0), stop=False)
                    for pair in range(2):
                        pc = slice(pair * 128, (pair + 1) * 128)
                        for j in range(NJ):
                            js = slice(j * 512, (j + 1) * 512)
                            ps = pss[pair * NJ + j]
                            nc.tensor.matmul(ps, b_sb[:, pc], ones[:, 0:512],
                                             start=False, stop=True)
                            nc.vector.tensor_copy(dst[:, pair, js], ps)

                # vh in natural [s, d] layout (+ ones column already set)
                with tc.tile_pool(name="ppv", bufs=3, space="PSUM") as ppv:
                  for st in range(QT):
                    ss = slice(st * 128, (st + 1) * 128)
                    ps = ppv.tile([128, HPC * DK], f32, tag="vp")
                    for c in range(EC):
                        nc.tensor.matmul(ps, vT_sb[:, c, ss], wv_sb[:, c, :],
                                         start=(c == 0), stop=False)
                    nc.tensor.matmul(ps, ones[:, 0:128], bv_sb[:, :],
                                     start=False, stop=True)
                    nc.vector.tensor_copy(
                        vh[:, st, :, 0:DK],
                        ps.rearrange("p (h d) -> p h d", h=HPC))

            # ---------------- phases A and B per head pair ----------------
            for pair in range(2):
                # phase A: scores [q, k], exp + sums, scale, writeout
                with tc.tile_pool(name=f"scA{pair}", bufs=2, space="PSUM") as scA, \
                     tc.tile_pool(name=f"exA{pair}", bufs=3) as exA, \
                     tc.tile_pool(name=f"wrA{pair}", bufs=3) as wrA, \
                     tc.tile_pool(name=f"rcA{pair}", bufs=8) as rcA:
                    for qt in range(QT):
                        qs = slice(qt * 128, (qt + 1) * 128)
                        pss = [scA.tile([128, s], f32, tag="sc",
                                        name=f"sc{qt}_{hh}")
                               for hh in range(2)]
                        for j in range(NJ):
                            js = slice(j * 512, (j + 1) * 512)
                            for hh in range(2):
                                hp = slice(hh * 64, (hh + 1) * 64)
                                nc.tensor.matmul(
                                    pss[hh][:, js],
                                    qhT[hp, pair, qs], khT[hp, pair, js],
                                    start=True, stop=True)
                        if qt == 0:
                            warm(pss[1], 8, 512)
                        for hh in range(2):
                            h = 2 * pair + hh
                            ex = exA.tile([128, s], f32, tag="ex")
                            sm = rcA.tile([128, 1], f32, tag="sm")
                            nc.scalar.activation(ex, pss[hh], AF.Exp,
                                                 accum_out=sm)
                            rc = rcA.tile([128, 1], f32, tag="rc")
                            nc.vector.reciprocal_approx_fast(rc, sm)
                            wr = wrA.tile([128, s], bf16, tag="wr")
                            nc.vector.tensor_scalar_mul(wr, ex, rc)
                            nc.sync.dma_start(attn_d[h, qs, :], wr)

                # phase B: scores^T [k, q], exp, PV with sums row, ctxT
                with tc.tile_pool(name=f"scB{pair}", bufs=2, space="PSUM") as scB, \
                     tc.tile_pool(name=f"pvB{pair}", bufs=4 * J2, space="PSUM") as pvB, \
                     tc.tile_pool(name=f"exB{pair}", bufs=4) as exB, \
                     tc.tile_pool(name=f"rbB{pair}", bufs=4) as rbB:
                    for hf in range(NH):
                        pv = [[[pvB.tile([DK + 1, 512], f32, tag="pv",
                                         name=f"pv{hf}_{hh}_{half}_{j2}")
                                for j2 in range(J2)] for half in range(2)]
                              for hh in range(2)]

                        # software-pipelined emission: PE's strict FIFO must
                        # never block on an exp -- scores^T mms for k-tile kt
                        # are emitted before the PV mms of k-tile kt-1, and
                        # the two heads' mms are adjacent (disjoint PE
                        # row-groups run concurrently).
                        def _emit_pv(kt):
                            # contraction split into 64-row halves: the four
                            # matmuls per k-tile alternate PE row-groups and
                            # write four DISTINCT PSUM banks, so they run
                            # concurrently (row-group packing) and LDWEIGHTS
                            # pulls ahead. Halves are merged at copy-out.
                            for j2 in range(J2):
                                js = slice(j2 * 512, (j2 + 1) * 512)
                                for step in range(2):
                                    for hh in range(2):
                                        half = step ^ hh
                                        rp = slice(half * 64, (half + 1) * 64)
                                        h = 2 * pair + hh
                                        nc.tensor.matmul(
                                            pv[hh][half][j2],
                                            vh[rp, kt, h, :],
                                            exs[kt % 2][hh][rp, js],
                                            start=(kt == 0),
                                            stop=(kt == QT - 1))

                        exs = [None, None]
                        for kt in range(QT):
                            ks = slice(kt * 128, (kt + 1) * 128)
                            # both heads share one [128, 1024] tile: the two
                            # row-group-packed matmuls land in different PSUM
                            # banks (cols 0-511 / 512-1023) and stay
                            # concurrent, and ONE exp covers both heads.
                            scT = scB.tile([128, 2 * WB], f32, tag="scT",
                                           name=f"scT{hf}_{kt}")
                            qcol = slice(hf * WB, (hf + 1) * WB)
                            for hh in range(2):
                                hp = slice(hh * 64, (hh + 1) * 64)
                                nc.tensor.matmul(
                                    scT[:, hh * WB : (hh + 1) * WB],
                                    khT[hp, pair, ks], qhT[hp, pair, qcol],
                                    start=True, stop=True)
                            if kt == 0 and hf == 0:
                                warm(scT, 8, 512)
                            if kt > 0:
                                _emit_pv(kt - 1)
                            ex = exB.tile([128, 2 * WB], bf16, tag="ex",
                                          name=f"ex{hf}_{kt}")
                            nc.scalar.activation(ex, scT, AF.Exp)
                            exs[kt % 2] = [ex[:, 0:WB], ex[:, WB : 2 * WB]]
                        _emit_pv(QT - 1)

                        # copy PV accumulators out of PSUM promptly (frees the
                        # banks for the next slab), then normalize from SBUF
                        for hh in range(2):
                            hp = slice(hh * 64, (hh + 1) * 64)
                            for j2 in range(J2):
                                qcol = slice(hf * WB + j2 * 512,
                                             hf * WB + (j2 + 1) * 512)
                                h0c = rbB.tile([DK + 1, 512], f32,
                                               tag="h0c")
                                nc.vector.tensor_copy(h0c, pv[hh][0][j2])
                                cp = rbB.tile([DK, 512], f32, tag="cp")
                                nc.vector.tensor_add(
                                    cp, h0c[0:DK, :], pv[hh][1][j2][0:DK, :])
                                srow = rbB.tile([1, 512], f32, tag="srow")
                                nc.vector.tensor_add(
                                    srow, h0c[DK : DK + 1, :],
                                    pv[hh][1][j2][DK : DK + 1, :])
                                sb = rbB.tile([DK, 512], f32, tag="sb")
                                nc.gpsimd.partition_broadcast(sb, srow)
                                rb = rbB.tile([DK, 512], f32, tag="rb")
                                nc.vector.reciprocal_approx_fast(rb, sb)
                                nc.vector.tensor_mul(
                                    ctxT[hp, pair, qcol], cp, rb)

            # ---------------- phase O: output projection ----------------
            with tc.tile_pool(name="oP", bufs=4, space="PSUM") as oP, \
                 tc.tile_pool(name="oS", bufs=4) as oS:
                for st in range(QT):
                    ss = slice(st * 128, (st + 1) * 128)
                    for ej in range(D // 512):
                        es = slice(ej * 512, (ej + 1) * 512)
                        po = oP.tile([128, 512], f32, tag="po")
                        for c in range(2):
                            nc.tensor.matmul(po, ctxT[:, c, ss],
                                             wo_sb[:, c, es],
                                             start=(c == 0), stop=(c == 1))
                        ob = oS.tile([128, 512], bf16, tag="ob")
                        nc.vector.tensor_copy(ob, po)
                        nc.sync.dma_start(outp_d[ss, es], ob)

    nc.compile()
    return nc


def _get_nc(s=S):
    if s not in _CACHE:
        _CACHE[s] = _build_nc(s)
    return _CACHE[s]


def kernel(q, k, v, W_q, b_q, W_k, b_k, W_v, b_v, W_o, b_o):
    from concourse.bass_utils import run_bass_kernel_spmd

    q = np.asarray(q, np.float32)
    k = np.asarray(k, np.float32)
    v = np.asarray(v, np.float32)
    s = q.shape[1]
    nc = _get_nc(s)

    scale = np.float32(1.0 / np.sqrt(DK))
    # per-batch transposed activations (shared by the 4 cores of a batch)
    qT = [np.ascontiguousarray(q[b].T).astype(_BF16) for b in range(B)]
    kT = [np.ascontiguousarray(k[b].T).astype(_BF16) for b in range(B)]
    vT = [np.ascontiguousarray(v[b].T).astype(_BF16) for b in range(B)]

    in_maps = []
    for c in range(NCORES):
        b, hg = c // 4, c % 4
        cols = slice(hg * HPC * DK, (hg + 1) * HPC * DK)
        in_maps.append({
            "qT": qT[b], "kT": kT[b], "vT": vT[b],
            "wq": np.ascontiguousarray(W_q[:, cols] * scale).astype(_BF16),
            "wk": np.ascontiguousarray(W_k[:, cols]).astype(_BF16),
            "wv": np.ascontiguousarray(W_v[:, cols]).astype(_BF16),
            "wo": np.ascontiguousarray(W_o[cols, :]).astype(_BF16),
            "bq": (b_q[cols] * scale).astype(_BF16).reshape(1, -1),
            "bk": b_k[cols].astype(_BF16).reshape(1, -1),
            "bv": b_v[cols].astype(_BF16).reshape(1, -1),
        })

    res = run_bass_kernel_spmd(nc, in_maps, core_ids=list(range(NCORES)))

    attn = np.empty((B, H, s, s), np.float32)
    out = np.zeros((B, s, D), np.float32)
    for c in range(NCORES):
        b, hg = c // 4, c % 4
        attn[b, hg * HPC : (hg + 1) * HPC] = res.results[c]["attn"].astype(np.float32)
        out[b] += res.results[c]["outp"].astype(np.float32)
    out += b_o.astype(np.float32)
    return out, attn
